# revision 3
# baseline (speedup 1.0000x reference)
"""MultiHeadEMA on 8 Trainium2 NeuronCores.

Strategy
--------
Channel-sharded: embed_dim=1024 is split into 8 slices of 128 channels, one
per core (128 channels == 128 SBUF partitions). Everything (EMA kernel
synthesis, the causal conv, gating) is independent per channel.

The FFT conv in the reference is mathematically an order-2 IIR:
    out[b,d,l] = sum_n c_n[d] * y_n[b,d,l],
    y_n[l] = q_n * y_n[l-1] + x[l]      (q_n = 1 - sigmoid(delta)*sigmoid(alpha))
    c_n    = sigmoid(delta)*beta*gamma*scale
so each (n, batch) stream is one `tensor_tensor_scan` on the vector engine
(state = q*state + x along the free axis). The final combine
    pre = c0*y0 + c1*y1 + omega*x
runs on the tensor engine as 3 accumulating diagonal matmuls into PSUM, and
the scalar engine applies silu while evacuating PSUM -> SBUF.

Host side only reshapes: per-core x slice is transposed to [128, B, L] so all
DMAs are fully contiguous (the kernel itself is memory-bound).
"""

import numpy as np

import concourse.bass as bass
import concourse.bacc as bacc
import concourse.tile as tile
from concourse import mybir
from concourse.bass_utils import run_bass_kernel_spmd

SEQ_LEN, BSZ, EMBED_DIM, NDIM = 4096, 4, 1024, 2
N_CORES = 8
D_PER = EMBED_DIM // N_CORES  # 128 channels/core = full SBUF partitions
SCALE = (1.0 / NDIM) ** 0.5
MM_CHUNK = 512  # fp32 PSUM bank limit
F32 = mybir.dt.float32
AF = mybir.ActivationFunctionType
ALU = mybir.AluOpType


def build_bass():
    nc = bacc.Bacc(name="multihead_ema")
    x = nc.dram_tensor("x", [D_PER, BSZ, SEQ_LEN], F32, kind="ExternalInput")
    # coef columns: [delta0, delta1, alpha0, alpha1, beta0, beta1, gamma0, gamma1, omega]
    coef = nc.dram_tensor("coef", [D_PER, 9], F32, kind="ExternalInput")
    eye = nc.dram_tensor("eye", [D_PER, D_PER], F32, kind="ExternalInput")
    out = nc.dram_tensor("out", [D_PER, BSZ, SEQ_LEN], F32, kind="ExternalOutput")

    with tile.TileContext(nc) as tc:
        with (
            tc.tile_pool(name="const", bufs=1) as const,
            tc.tile_pool(name="xp", bufs=2) as xp,
            tc.tile_pool(name="y0p", bufs=2) as y0p,
            tc.tile_pool(name="y1p", bufs=2) as y1p,
            tc.tile_pool(name="op", bufs=3) as op,
            tc.tile_pool(name="sgp", bufs=4) as sgp,
            tc.tile_pool(name="tpp", bufs=4) as tpp,
            tc.tile_pool(name="ps", bufs=8, space="PSUM") as ps,
        ):
            csb = const.tile([D_PER, 9], F32)
            nc.sync.dma_start(out=csb[:, :], in_=coef[:, :])
            eyesb = const.tile([D_PER, D_PER], F32)
            nc.sync.dma_start(out=eyesb[:, :], in_=eye[:, :])

            # p = sigmoid(delta), sa = sigmoid(alpha)  (packed as 4 cols)
            sig = const.tile([D_PER, 4], F32)
            nc.scalar.activation(out=sig[:, :], in_=csb[:, 0:4], func=AF.Sigmoid)
            # q = 1 - p*sa
            pq = const.tile([D_PER, NDIM], F32)
            nc.vector.tensor_mul(out=pq[:, :], in0=sig[:, 0:2], in1=sig[:, 2:4])
            q = const.tile([D_PER, NDIM], F32)
            nc.scalar.activation(
                out=q[:, :], in_=pq[:, :], func=AF.Copy, scale=-1.0, bias=1.0
            )
            # c = p * beta * gamma * SCALE
            c1t = const.tile([D_PER, NDIM], F32)
            nc.vector.tensor_mul(out=c1t[:, :], in0=sig[:, 0:2], in1=csb[:, 4:6])
            c2t = const.tile([D_PER, NDIM], F32)
            nc.vector.tensor_mul(out=c2t[:, :], in0=c1t[:, :], in1=csb[:, 6:8])
            cc = const.tile([D_PER, NDIM], F32)
            nc.scalar.mul(out=cc[:, :], in_=c2t[:, :], mul=SCALE)

            # diagonal weight matrices for the PE combine
            dc0 = const.tile([D_PER, D_PER], F32)
            nc.vector.tensor_scalar_mul(out=dc0[:, :], in0=eyesb[:, :], scalar1=cc[:, 0:1])
            dc1 = const.tile([D_PER, D_PER], F32)
            nc.vector.tensor_scalar_mul(out=dc1[:, :], in0=eyesb[:, :], scalar1=cc[:, 1:2])
            dw = const.tile([D_PER, D_PER], F32)
            nc.vector.tensor_scalar_mul(out=dw[:, :], in0=eyesb[:, :], scalar1=csb[:, 8:9])

            q0b = q[:, 0:1].to_broadcast([D_PER, SEQ_LEN])
            q1b = q[:, 1:2].to_broadcast([D_PER, SEQ_LEN])

            for b in range(BSZ):
                xb = xp.tile([D_PER, SEQ_LEN], F32)
                nc.sync.dma_start(out=xb[:, :], in_=x[:, b, :])
                y0 = y0p.tile([D_PER, SEQ_LEN], F32)
                nc.vector.tensor_tensor_scan(
                    out=y0[:, :], data0=q0b, data1=xb[:, :],
                    initial=0.0, op0=ALU.mult, op1=ALU.add,
                )
                y1 = y1p.tile([D_PER, SEQ_LEN], F32)
                nc.vector.tensor_tensor_scan(
                    out=y1[:, :], data0=q1b, data1=xb[:, :],
                    initial=0.0, op0=ALU.mult, op1=ALU.add,
                )
                ob = op.tile([D_PER, SEQ_LEN], F32)
                for ci in range(SEQ_LEN // MM_CHUNK):
                    sl = bass.ts(ci, MM_CHUNK)
                    pt = ps.tile([D_PER, MM_CHUNK], F32)
                    nc.tensor.matmul(pt[:, :], dc0[:, :], y0[:, sl], start=True, stop=False)
                    nc.tensor.matmul(pt[:, :], dc1[:, :], y1[:, sl], start=False, stop=False)
                    nc.tensor.matmul(pt[:, :], dw[:, :], xb[:, sl], start=False, stop=True)
                    # silu(t) = t * sigmoid(t); ACT evacuates PSUM (sigmoid +
                    # copy), POOL does the multiply so DVE stays scan-only.
                    sg = sgp.tile([D_PER, MM_CHUNK], F32)
                    nc.scalar.activation(out=sg[:, :], in_=pt[:, :], func=AF.Sigmoid)
                    tcp = tpp.tile([D_PER, MM_CHUNK], F32)
                    nc.scalar.activation(out=tcp[:, :], in_=pt[:, :], func=AF.Copy)
                    nc.gpsimd.tensor_mul(out=ob[:, sl], in0=sg[:, :], in1=tcp[:, :])
                nc.sync.dma_start(out=out[:, b, :], in_=ob[:, :])

    nc.compile()
    return nc


_CACHE: dict = {}


def _get_nc():
    if "nc" not in _CACHE:
        _CACHE["nc"] = build_bass()
    return _CACHE["nc"]


def make_in_maps(inputs):
    x = np.asarray(inputs["x"], np.float32)
    delta = np.asarray(inputs["delta"], np.float32).reshape(EMBED_DIM, NDIM)
    alpha = np.asarray(inputs["alpha"], np.float32).reshape(EMBED_DIM, NDIM)
    beta = np.asarray(inputs["beta"], np.float32).reshape(EMBED_DIM, NDIM)
    gamma = np.asarray(inputs["gamma"], np.float32).reshape(EMBED_DIM, NDIM)
    omega = np.asarray(inputs["omega"], np.float32).reshape(EMBED_DIM, 1)
    coef_full = np.concatenate([delta, alpha, beta, gamma, omega], axis=1)
    eye = np.eye(D_PER, dtype=np.float32)
    in_maps = []
    for c in range(N_CORES):
        sl = slice(c * D_PER, (c + 1) * D_PER)
        xc = np.ascontiguousarray(x[:, :, sl].transpose(2, 1, 0))
        in_maps.append(
            {"x": xc, "coef": np.ascontiguousarray(coef_full[sl]), "eye": eye}
        )
    return in_maps


def gather_out(results):
    out = np.empty((SEQ_LEN, BSZ, EMBED_DIM), np.float32)
    for c in range(N_CORES):
        out[:, :, c * D_PER : (c + 1) * D_PER] = results[c]["out"].transpose(2, 1, 0)
    return out


def _run(inputs, **kwargs):
    nc = _get_nc()
    in_maps = make_in_maps(inputs)
    res = run_bass_kernel_spmd(nc, in_maps, core_ids=list(range(N_CORES)), **kwargs)
    return gather_out(res.results), res


def kernel(**inputs) -> np.ndarray:
    out, _ = _run(inputs)
    return out


# revision 5
# speedup vs baseline: 1.8721x; 1.8721x over previous
"""MultiHeadEMA on 8 Trainium2 NeuronCores.

Strategy
--------
Channel-sharded: embed_dim=1024 -> 8 slices of 128 channels (= SBUF
partitions), one per core. The reference's FFT conv is exactly an order-2 IIR
    y_n[l] = q_n y_n[l-1] + x[l],   out = silu(c0 y0 + c1 y1 + omega x)
so the conv becomes `tensor_tensor_scan` streams on the vector engine.

The DVE scan runs at ~2.1 cyc/elem, so the scan is decimated by 2: with
    u_n[m] = x[2m] + q_n x[2m-1]           (tensor engine, diagonal matmuls)
    y_n[2m] = q_n^2 y_n[2m-2] + u_n[m]     (DVE scan at half length, from PSUM)
    y_n[2m+1] = q_n y_n[2m] + x[2m+1]      (folded into the output matmuls)
Even/odd outputs are per-channel linear combos of (y0_even, y1_even, x):
    pre_even = c0 y0e + c1 y1e + w x_even
    pre_odd  = (c0 q0) y0e + (c1 q1) y1e + (c0+c1+w) x_odd
computed as accumulating diagonal matmuls into PSUM (bf16 weights/rhs), then
one Silu activation per chunk evacuates PSUM -> SBUF with an interleaved
(stride-2) write. Interior math is bf16 (fp32 PSUM accumulation and fp32 scan
state); decay factors q, q^2 stay exact fp32.

Host side only reshapes/casts: per-core x slice is transposed to [128, B, L],
cast to bf16, and deinterleaved into x_even / x_odd / x_oshift (x_oshift[m] =
x[2m-1], zero-padded) so every device access pattern is contiguous.
"""

import numpy as np
import ml_dtypes

import concourse.bass as bass
import concourse.bacc as bacc
import concourse.tile as tile
from concourse import mybir
from concourse.bass_utils import run_bass_kernel_spmd

SEQ_LEN, BSZ, EMBED_DIM, NDIM = 4096, 4, 1024, 2
N_CORES = 8
D_PER = EMBED_DIM // N_CORES  # 128 channels/core = full SBUF partitions
SCALE = (1.0 / NDIM) ** 0.5
M = SEQ_LEN // 2          # decimated length 2048
CH = 512                  # matmul/psum chunk (one fp32 PSUM bank)
NCH = M // CH             # 4 chunks per slab
UP = 1024                 # scan piece (2 PSUM banks)
F32 = mybir.dt.float32
BF16 = mybir.dt.bfloat16
AF = mybir.ActivationFunctionType
ALU = mybir.AluOpType


def build_bass():
    nc = bacc.Bacc(name="multihead_ema")
    xe = nc.dram_tensor("x_even", [D_PER, BSZ, M], BF16, kind="ExternalInput")
    xo = nc.dram_tensor("x_odd", [D_PER, BSZ, M], BF16, kind="ExternalInput")
    xs = nc.dram_tensor("x_oshift", [D_PER, BSZ, M], BF16, kind="ExternalInput")
    # coef columns: [delta0, delta1, alpha0, alpha1, beta0, beta1, gamma0, gamma1, omega]
    coef = nc.dram_tensor("coef", [D_PER, 9], F32, kind="ExternalInput")
    eye = nc.dram_tensor("eye", [D_PER, D_PER], BF16, kind="ExternalInput")
    out = nc.dram_tensor("out", [D_PER, BSZ, SEQ_LEN], BF16, kind="ExternalOutput")

    with tile.TileContext(nc) as tc:
        with (
            tc.tile_pool(name="const", bufs=1) as const,
            tc.tile_pool(name="xep", bufs=3) as xep,
            tc.tile_pool(name="xop", bufs=3) as xop,
            tc.tile_pool(name="xsp", bufs=3) as xsp,
            tc.tile_pool(name="yp", bufs=2) as yp,
            tc.tile_pool(name="op", bufs=3) as op,
            tc.tile_pool(name="psu", bufs=3, space="PSUM") as psu,
            tc.tile_pool(name="psc", bufs=2, space="PSUM") as psc,
        ):
            csb = const.tile([D_PER, 9], F32)
            nc.sync.dma_start(out=csb[:, :], in_=coef[:, :])
            eyesb = const.tile([D_PER, D_PER], BF16)
            nc.sync.dma_start(out=eyesb[:, :], in_=eye[:, :])

            # --- per-channel coefficients (all [128, 1 or 2] fp32, trivial)
            sig = const.tile([D_PER, 4], F32)  # [p0, p1, sa0, sa1]
            nc.scalar.activation(out=sig[:, :], in_=csb[:, 0:4], func=AF.Sigmoid)
            pq = const.tile([D_PER, NDIM], F32)
            nc.vector.tensor_mul(out=pq[:, :], in0=sig[:, 0:2], in1=sig[:, 2:4])
            q = const.tile([D_PER, NDIM], F32)  # q = 1 - p*sigmoid(alpha)
            nc.scalar.activation(out=q[:, :], in_=pq[:, :], func=AF.Copy,
                                 scale=-1.0, bias=1.0)
            q2 = const.tile([D_PER, NDIM], F32)
            nc.vector.tensor_mul(out=q2[:, :], in0=q[:, :], in1=q[:, :])
            c1t = const.tile([D_PER, NDIM], F32)
            nc.vector.tensor_mul(out=c1t[:, :], in0=sig[:, 0:2], in1=csb[:, 4:6])
            c2t = const.tile([D_PER, NDIM], F32)
            nc.vector.tensor_mul(out=c2t[:, :], in0=c1t[:, :], in1=csb[:, 6:8])
            cc = const.tile([D_PER, NDIM], F32)  # c_n = p beta gamma scale
            nc.scalar.mul(out=cc[:, :], in_=c2t[:, :], mul=SCALE)
            cq = const.tile([D_PER, NDIM], F32)  # c_n * q_n
            nc.vector.tensor_mul(out=cq[:, :], in0=cc[:, :], in1=q[:, :])
            csum = const.tile([D_PER, 1], F32)   # c0 + c1 + w
            nc.vector.tensor_add(out=csum[:, :], in0=cc[:, 0:1], in1=cc[:, 1:2])
            nc.vector.tensor_add(out=csum[:, :], in0=csum[:, :], in1=csb[:, 8:9])

            # --- bf16 diagonal weight matrices for the PE
            _diag_n = [0]

            def diag(scalar_ap):
                _diag_n[0] += 1
                t = const.tile([D_PER, D_PER], BF16, tag=f"diag{_diag_n[0]}")
                nc.vector.tensor_scalar_mul(out=t[:, :], in0=eyesb[:, :],
                                            scalar1=scalar_ap)
                return t

            w_q = [diag(q[:, n : n + 1]) for n in range(NDIM)]
            w_c = [diag(cc[:, n : n + 1]) for n in range(NDIM)]
            w_cq = [diag(cq[:, n : n + 1]) for n in range(NDIM)]
            w_w = diag(csb[:, 8:9])
            w_cs = diag(csum[:, 0:1])

            q2b = [q2[:, n : n + 1].to_broadcast([D_PER, UP]) for n in range(NDIM)]

            for b in range(BSZ):
                xeb = xep.tile([D_PER, M], BF16)
                nc.sync.dma_start(out=xeb[:, :], in_=xe[:, b, :])
                xob = xop.tile([D_PER, M], BF16)
                nc.sync.dma_start(out=xob[:, :], in_=xo[:, b, :])
                xsb = xsp.tile([D_PER, M], BF16)
                nc.sync.dma_start(out=xsb[:, :], in_=xs[:, b, :])

                # --- u_n = x_even + q_n * x_oshift (PE, PSUM) ; y_n = scan(u_n)
                y = []
                for n in range(NDIM):
                    yn = yp.tile([D_PER, M], BF16, tag=f"y{n}")
                    for p in range(M // UP):
                        pu = psu.tile([D_PER, UP], F32, tag="u")
                        for h in range(UP // CH):
                            s = bass.ts(p * (UP // CH) + h, CH)
                            nc.tensor.matmul(pu[:, bass.ts(h, CH)], eyesb[:, :],
                                             xeb[:, s], start=True, stop=False)
                            nc.tensor.matmul(pu[:, bass.ts(h, CH)], w_q[n][:, :],
                                             xsb[:, s], start=False, stop=True)
                        init = 0.0 if p == 0 else yn[:, p * UP - 1 : p * UP]
                        nc.vector.tensor_tensor_scan(
                            out=yn[:, bass.ts(p, UP)], data0=q2b[n], data1=pu[:, :],
                            initial=init, op0=ALU.mult, op1=ALU.add,
                        )
                    y.append(yn)

                # --- outputs: interleaved silu(pre) straight out of PSUM
                ob = op.tile([D_PER, SEQ_LEN], BF16)
                for ci in range(NCH):
                    s = bass.ts(ci, CH)
                    pe = psc.tile([D_PER, CH], F32, tag="cmb")
                    nc.tensor.matmul(pe[:, :], w_c[0][:, :], y[0][:, s], start=True, stop=False)
                    nc.tensor.matmul(pe[:, :], w_c[1][:, :], y[1][:, s], start=False, stop=False)
                    nc.tensor.matmul(pe[:, :], w_w[:, :], xeb[:, s], start=False, stop=True)
                    nc.scalar.activation(
                        out=ob[:, 2 * CH * ci : 2 * CH * (ci + 1) : 2],
                        in_=pe[:, :], func=AF.Silu,
                    )
                    po = psc.tile([D_PER, CH], F32, tag="cmb")
                    nc.tensor.matmul(po[:, :], w_cq[0][:, :], y[0][:, s], start=True, stop=False)
                    nc.tensor.matmul(po[:, :], w_cq[1][:, :], y[1][:, s], start=False, stop=False)
                    nc.tensor.matmul(po[:, :], w_cs[:, :], xob[:, s], start=False, stop=True)
                    nc.scalar.activation(
                        out=ob[:, 2 * CH * ci + 1 : 2 * CH * (ci + 1) : 2],
                        in_=po[:, :], func=AF.Silu,
                    )
                nc.sync.dma_start(out=out[:, b, :], in_=ob[:, :])

    nc.compile()
    return nc


_CACHE: dict = {}


def _get_nc():
    if "nc" not in _CACHE:
        _CACHE["nc"] = build_bass()
    return _CACHE["nc"]


def make_in_maps(inputs):
    x = np.asarray(inputs["x"], np.float32)
    delta = np.asarray(inputs["delta"], np.float32).reshape(EMBED_DIM, NDIM)
    alpha = np.asarray(inputs["alpha"], np.float32).reshape(EMBED_DIM, NDIM)
    beta = np.asarray(inputs["beta"], np.float32).reshape(EMBED_DIM, NDIM)
    gamma = np.asarray(inputs["gamma"], np.float32).reshape(EMBED_DIM, NDIM)
    omega = np.asarray(inputs["omega"], np.float32).reshape(EMBED_DIM, 1)
    coef_full = np.concatenate([delta, alpha, beta, gamma, omega], axis=1)
    eye = np.eye(D_PER, dtype=ml_dtypes.bfloat16)
    in_maps = []
    for c in range(N_CORES):
        sl = slice(c * D_PER, (c + 1) * D_PER)
        xc = x[:, :, sl].transpose(2, 1, 0).astype(ml_dtypes.bfloat16)  # [128,B,L]
        x_even = np.ascontiguousarray(xc[:, :, 0::2])
        x_odd = np.ascontiguousarray(xc[:, :, 1::2])
        x_oshift = np.zeros_like(x_odd)
        x_oshift[:, :, 1:] = x_odd[:, :, :-1]
        in_maps.append(
            {
                "x_even": x_even,
                "x_odd": x_odd,
                "x_oshift": x_oshift,
                "coef": np.ascontiguousarray(coef_full[sl]),
                "eye": eye,
            }
        )
    return in_maps


def gather_out(results):
    out = np.empty((SEQ_LEN, BSZ, EMBED_DIM), np.float32)
    for c in range(N_CORES):
        out[:, :, c * D_PER : (c + 1) * D_PER] = (
            results[c]["out"].astype(np.float32).transpose(2, 1, 0)
        )
    return out


def _run(inputs, **kwargs):
    nc = _get_nc()
    in_maps = make_in_maps(inputs)
    res = run_bass_kernel_spmd(nc, in_maps, core_ids=list(range(N_CORES)), **kwargs)
    return gather_out(res.results), res


def kernel(**inputs) -> np.ndarray:
    out, _ = _run(inputs)
    return out
